# revision 1
# baseline (speedup 1.0000x reference)
"""Block-sparse 3-layer MLP on 8 Trainium2 NeuronCores.

Reference computation (fp32):
    h1 = relu(x @ (W1*expand(mask1)).T + b1)       x:[B,2048] W1:[4096,2048]
    h2 = relu(h1 @ (W2*expand(mask2)).T + b2)      W2:[4096,4096]
    out = h2 @ Wo.T + bo                           Wo:[1024,4096] -> [B,1024]

Strategy: data-parallel over the batch (B=8192 -> 1024 rows/core), no
collectives. Masks are applied to the weights on the host (free), and all
matmuls run dense on the PE array. Activations are kept feature-major
[features, batch] on-chip so biases are per-partition and `lhsT` panels are
pre-transposed on the host into contiguous [128, K] blocks.

Per core:
  L1: h1 (32 tiles [128,1024]) stays resident in SBUF.
  L2+L3 fused: for each of W2's 32 row-blocks, compute h2 tile, immediately
  multiply against Wo panels, accumulate the [1024,1024] output in SBUF via
  DVE adds. No intermediate ever touches DRAM; W1/W2/Wo are each read once.

MM_DTYPE selects the PE datapath: fp32 is exact but 4 cycles/row on the PE;
bf16 and float32r stream at 1 cycle/row (4x faster).
"""

import sys

sys.path.insert(0, "/opt/trn_rl_repo")

import numpy as np

from concourse import bacc, mybir, tile
import concourse.bass_utils as _bass_utils
from concourse.bass_utils import run_bass_kernel_spmd

# Walrus is invoked with --enable-ldw-opt=false, which emits one LDWEIGHTS
# per matmul even when consecutive matmuls share the stationary operand.
# Measured on HW: deduping LDWEIGHTS makes the kernel SLOWER (234 vs 227
# ns/matmul) — the per-matmul LDWEIGHTS stream is what keeps the background
# weight buffer fed for pull-ahead. Keep this off.
LDW_OPT = False

if LDW_OPT and not hasattr(_bass_utils, "_orig_run_command_ldw"):
    _bass_utils._orig_run_command_ldw = _bass_utils.run_command

    def _run_command_ldw(cmd, *a, **kw):
        if isinstance(cmd, list):
            cmd = [
                "--enable-ldw-opt=true" if c == "--enable-ldw-opt=false" else c
                for c in cmd
            ]
        return _bass_utils._orig_run_command_ldw(cmd, *a, **kw)

    _bass_utils.run_command = _run_command_ldw

F32 = mybir.dt.float32
F32R = mybir.dt.float32r
BF16 = mybir.dt.bfloat16
RELU = mybir.ActivationFunctionType.Relu
IDENT = mybir.ActivationFunctionType.Identity

N_CORES = 8
TILE = 32  # block-sparse tile size of the masks
P = 128  # partitions

MM_DTYPE = "f32r"  # "f32" | "f32r" | "bf16"


def _build(nc, d_in, d_h, d_out, bc, mm_dtype=MM_DTYPE):
    """Emit the per-core kernel. bc = batch columns per core."""
    kt1 = d_in // P  # k-tiles in layer 1
    mt1 = d_h // P  # m-tiles of h1 (== k-tiles of layer 2)
    mt2 = d_h // P  # m-tiles of h2
    mot = d_out // P  # m-tiles of out
    sw = min(512, bc)  # psum strip width
    ns = bc // sw  # strips per row of tiles

    # storage dtype of mm operands
    sdt = {"bf16": BF16, "f32r": F32R, "f32": F32}[mm_dtype]

    def mm(out_ps, lhsT, rhs, start, stop):
        nc.tensor.matmul(out_ps, lhsT, rhs, start=start, stop=stop)

    xt_d = nc.dram_tensor("xt", [kt1, P, bc], sdt, kind="ExternalInput")
    w1_d = nc.dram_tensor("w1", [mt1, P, d_in], sdt, kind="ExternalInput")
    b1_d = nc.dram_tensor("b1", [P, mt1], F32, kind="ExternalInput")
    w2_d = nc.dram_tensor("w2", [mt2, P, d_h], sdt, kind="ExternalInput")
    b2_d = nc.dram_tensor("b2", [P, mt2], F32, kind="ExternalInput")
    wo_d = nc.dram_tensor("wo", [mt2, P, d_out], sdt, kind="ExternalInput")
    bo_d = nc.dram_tensor("bo", [P, mot], F32, kind="ExternalInput")
    out_d = nc.dram_tensor("out", [mot, P, bc], F32, kind="ExternalOutput")

    with tile.TileContext(nc) as tc:
        with (
            tc.tile_pool(name="bias", bufs=1) as bias_pool,
            tc.tile_pool(name="h1", bufs=1) as h1_pool,
        ):
            b1_sb = bias_pool.tile([P, mt1], F32, tag="b1")
            b2_sb = bias_pool.tile([P, mt2], F32, tag="b2")
            bo_sb = bias_pool.tile([P, mot], F32, tag="bo")
            nc.sync.dma_start(out=b1_sb[:], in_=b1_d[:])
            nc.sync.dma_start(out=b2_sb[:], in_=b2_d[:])
            nc.sync.dma_start(out=bo_sb[:], in_=bo_d[:])

            h1 = []
            for i in range(mt1):
                t = h1_pool.tile([P, bc], sdt, name=f"h1_{i}", tag=f"h1_{i}")
                h1.append(t)

            kh2 = max(mt1 // 8, 1)  # k-tiles per w2 panel piece

            # ps2 lives above the L1 scope so layer 2's first matmuls are not
            # gated on ps1's pool release; w2pre holds the first two pieces of
            # w2's first panel, loaded while layer 1 is still running.
            with (
                tc.tile_pool(name="ps2", bufs=2, space="PSUM") as ps2_pool,
                tc.tile_pool(name="w2pre", bufs=2) as w2pre_pool,
            ):
                # ---------------- Layer 1 ----------------
                with (
                    tc.tile_pool(name="xtp", bufs=1) as xt_pool,
                    tc.tile_pool(name="w1p", bufs=4) as w1_pool,
                    tc.tile_pool(name="ps1", bufs=2, space="PSUM") as ps1_pool,
                ):
                    # stream each weight panel in quarters so the pool stays
                    # small enough to double-buffer within SBUF
                    kh1 = max(kt1 // 4, 1)  # k-tiles per panel piece

                    def load_w1(mt, eng=None):
                        w1h = []
                        for h in range(kt1 // kh1):
                            t = w1_pool.tile([P, kh1 * P], sdt, tag="w1t")
                            (eng or nc.sync).dma_start(
                                out=t[:],
                                in_=w1_d[mt][:, h * kh1 * P : (h + 1) * kh1 * P],
                            )
                            w1h.append(t)
                        return w1h

                    # xt alternates between the sync and gpsimd DMA rings for
                    # 2x startup bandwidth; the first two tiles go at the very
                    # head of each ring so the PE's first matmuls are gated
                    # only on xt_0 + the first w1 piece
                    xt = []
                    for kt in range(kt1):
                        t = xt_pool.tile([P, bc], sdt, name=f"xt_{kt}", tag=f"xt_{kt}")
                        xt.append(t)
                    nc.sync.dma_start(out=xt[0][:], in_=xt_d[0])
                    if kt1 > 1:
                        nc.gpsimd.dma_start(out=xt[1][:], in_=xt_d[1])
                    w1h0 = load_w1(0)
                    for kt in range(2, kt1):
                        eng = nc.sync if kt % 2 == 0 else nc.gpsimd
                        eng.dma_start(out=xt[kt][:], in_=xt_d[kt])

                    w2pre = []
                    for h in range(min(2, mt1 // kh2)):
                        t = w2pre_pool.tile([P, kh2 * P], sdt, tag="w2pre")
                        nc.sync.dma_start(
                            out=t[:], in_=w2_d[0][:, h * kh2 * P : (h + 1) * kh2 * P]
                        )
                        w2pre.append(t)

                    for mt in range(mt1):
                        w1h = w1h0 if mt == 0 else load_w1(mt)
                        # weight-stationary inner order: each lhsT block feeds
                        # every batch strip before the next LDWEIGHTS; the
                        # psum tile spans ns banks so one activation drains it
                        ps = ps1_pool.tile([P, bc], F32, name="ps1", tag="ps1")
                        for kt in range(kt1):
                            h, r = divmod(kt, kh1)
                            for n in range(ns):
                                mm(
                                    ps[:, n * sw : (n + 1) * sw],
                                    w1h[h][:, r * P : (r + 1) * P],
                                    xt[kt][:, n * sw : (n + 1) * sw],
                                    start=(kt == 0),
                                    stop=(kt == kt1 - 1),
                                )
                        nc.scalar.activation(
                            h1[mt][:], ps[:], RELU, bias=b1_sb[:, mt : mt + 1]
                        )

                # ---------------- Layers 2 + 3 (fused) ----------------
                with (
                    tc.tile_pool(name="w2p", bufs=10) as w2_pool,
                    tc.tile_pool(name="wop", bufs=2) as wo_pool,
                    tc.tile_pool(name="h2p", bufs=2) as h2_pool,
                    tc.tile_pool(name="oacc", bufs=1) as oacc_pool,
                    tc.tile_pool(name="ps3", bufs=2, space="PSUM") as ps3_pool,
                ):
                    oacc = []
                    for i in range(mot):
                        t = oacc_pool.tile([P, bc], F32, name=f"oacc_{i}", tag=f"oacc_{i}")
                        oacc.append(t)

                    for mt in range(mt2):
                        if mt == 0:
                            w2h = list(w2pre)
                            hs = len(w2pre)
                        else:
                            w2h = []
                            hs = 0
                        for h in range(hs, mt1 // kh2):
                            t = w2_pool.tile([P, kh2 * P], sdt, tag="w2t")
                            nc.sync.dma_start(
                                out=t[:], in_=w2_d[mt][:, h * kh2 * P : (h + 1) * kh2 * P]
                            )
                            w2h.append(t)
                        wot = wo_pool.tile([P, d_out], sdt, tag="wot")
                        nc.sync.dma_start(out=wot[:], in_=wo_d[mt])
                        h2t = h2_pool.tile([P, bc], sdt, tag="h2t")
                        ps = ps2_pool.tile([P, bc], F32, name="ps2", tag="ps2")
                        for kt in range(mt1):
                            h, r = divmod(kt, kh2)
                            for n in range(ns):
                                mm(
                                    ps[:, n * sw : (n + 1) * sw],
                                    w2h[h][:, r * P : (r + 1) * P],
                                    h1[kt][:, n * sw : (n + 1) * sw],
                                    start=(kt == 0),
                                    stop=(kt == mt1 - 1),
                                )
                        nc.scalar.activation(
                            h2t[:], ps[:], RELU, bias=b2_sb[:, mt : mt + 1]
                        )
                        last = mt == mt2 - 1 and mt2 > 1
                        for mo in range(mot):
                            ps3 = ps3_pool.tile([P, bc], F32, name="ps3", tag="ps3")
                            for n in range(ns):
                                cs = slice(n * sw, (n + 1) * sw)
                                mm(
                                    ps3[:, cs],
                                    wot[:, mo * P : (mo + 1) * P],
                                    h2t[:, cs],
                                    start=True,
                                    stop=True,
                                )
                            if mt == 0:
                                nc.vector.tensor_copy(oacc[mo][:], ps3[:])
                            elif last:
                                # fold the output bias into the final
                                # accumulation: out = (ps3*1 + bo) + oacc
                                nc.vector.affine_then_add(
                                    oacc[mo][:],
                                    ps3[:],
                                    oacc[mo][:],
                                    1.0,
                                    bo_sb[:, mo : mo + 1],
                                )
                            else:
                                nc.vector.tensor_add(oacc[mo][:], oacc[mo][:], ps3[:])
                            if last:
                                nc.sync.dma_start(out=out_d[mo], in_=oacc[mo][:])

                    if mt2 == 1:
                        for mo in range(mot):
                            nc.scalar.activation(
                                oacc[mo][:], oacc[mo][:], IDENT, bias=bo_sb[:, mo : mo + 1]
                            )
                            nc.sync.dma_start(out=out_d[mo], in_=oacc[mo][:])

    nc.compile()
    return nc


def _expand_mask(mask, t=TILE):
    return np.repeat(np.repeat(np.asarray(mask, dtype=bool), t, axis=0), t, axis=1)


def _pack_lhsT(w, d_m, d_k):
    """[d_m, d_k] weights -> [d_m/P, P, d_k] panels.

    panel[mt, i, kt*P + j] = w[mt*P + j, kt*P + i], so each [P, P] slice of a
    panel is a ready-to-use lhsT block (partition dim = contraction dim).
    """
    mt, kt = d_m // P, d_k // P
    return np.ascontiguousarray(
        w.reshape(mt, P, kt, P).transpose(0, 3, 2, 1).reshape(mt, P, d_k)
    )


def _pack_out_panels(w, d_m, d_k):
    """[d_m, d_k] weights -> [d_k/P, P, d_m] panels keyed by the k-tile.

    panel[kt, i, mo*P + j] = w[mo*P + j, kt*P + i].
    """
    mt, kt = d_m // P, d_k // P
    return np.ascontiguousarray(
        w.reshape(mt, P, kt, P).transpose(2, 3, 0, 1).reshape(kt, P, d_m)
    )


def _pack_bias(b):
    n = b.shape[0] // P
    return np.ascontiguousarray(b.reshape(n, P).T)


def _run(
    x,
    w1e,
    b1,
    w2e,
    b2,
    wo,
    bo,
    d_in,
    d_h,
    d_out,
    n_cores=N_CORES,
    trace=False,
    mm_dtype=MM_DTYPE,
):
    b = x.shape[0]
    bc = b // n_cores

    nc = bacc.Bacc(
        "TRN2", target_bir_lowering=False, debug=False, num_devices=n_cores
    )
    _build(nc, d_in, d_h, d_out, bc, mm_dtype=mm_dtype)

    np_sdt = mybir.dt.np(BF16) if mm_dtype == "bf16" else np.float32

    def cvt(a):
        return np.ascontiguousarray(a.astype(np_sdt))

    shared = {
        "w1": cvt(_pack_lhsT(w1e, d_h, d_in)),
        "b1": _pack_bias(b1),
        "w2": cvt(_pack_lhsT(w2e, d_h, d_h)),
        "b2": _pack_bias(b2),
        "wo": cvt(_pack_out_panels(wo, d_out, d_h)),
        "bo": _pack_bias(bo),
    }
    in_maps = []
    for c in range(n_cores):
        xc = np.ascontiguousarray(x[c * bc : (c + 1) * bc].T).reshape(
            d_in // P, P, bc
        )
        in_maps.append({"xt": cvt(xc), **shared})

    res = run_bass_kernel_spmd(
        nc, in_maps, core_ids=list(range(n_cores)), trace=trace
    )
    outs = []
    for c in range(n_cores):
        outs.append(res.results[c]["out"].reshape(d_out, bc))
    full = np.concatenate(outs, axis=1)  # [d_out, B]
    return np.ascontiguousarray(full.T), res


def kernel(x, W1, b1, W2, b2, Wo, bo, mask1, mask2):
    x = np.asarray(x, dtype=np.float32)
    w1e = np.asarray(W1, dtype=np.float32) * _expand_mask(mask1)
    w2e = np.asarray(W2, dtype=np.float32) * _expand_mask(mask2)
    out, _ = _run(
        x,
        w1e,
        np.asarray(b1, np.float32),
        w2e,
        np.asarray(b2, np.float32),
        np.asarray(Wo, np.float32),
        np.asarray(bo, np.float32),
        d_in=2048,
        d_h=4096,
        d_out=1024,
    )
    return out



# revision 2
# speedup vs baseline: 1.0847x; 1.0847x over previous
"""Block-sparse 3-layer MLP on 8 Trainium2 NeuronCores.

Reference computation (fp32):
    h1 = relu(x @ (W1*expand(mask1)).T + b1)       x:[B,2048] W1:[4096,2048]
    h2 = relu(h1 @ (W2*expand(mask2)).T + b2)      W2:[4096,4096]
    out = h2 @ Wo.T + bo                           Wo:[1024,4096] -> [B,1024]

Strategy: data-parallel over the batch (B=8192 -> 1024 rows/core), no
collectives. Masks are applied to the weights on the host (free), and all
matmuls run dense on the PE array. Activations are kept feature-major
[features, batch] on-chip so biases are per-partition and `lhsT` panels are
pre-transposed on the host into contiguous [128, K] blocks.

Per core:
  L1: h1 (32 tiles [128,1024]) stays resident in SBUF.
  L2+L3 fused: for each of W2's 32 row-blocks, compute h2 tile, immediately
  multiply against Wo panels, accumulate the [1024,1024] output in SBUF via
  DVE adds. No intermediate ever touches DRAM; W1/W2/Wo are each read once.

MM_DTYPE selects the PE datapath: fp32 is exact but 4 cycles/row on the PE;
bf16 and float32r stream at 1 cycle/row (4x faster).
"""

import sys

sys.path.insert(0, "/opt/trn_rl_repo")

import numpy as np

from concourse import bacc, mybir, tile
import concourse.bass_utils as _bass_utils
from concourse.bass_utils import run_bass_kernel_spmd

# Walrus is invoked with --enable-ldw-opt=false, which emits one LDWEIGHTS
# per matmul even when consecutive matmuls share the stationary operand.
# Measured on HW: deduping LDWEIGHTS makes the kernel SLOWER (234 vs 227
# ns/matmul) — the per-matmul LDWEIGHTS stream is what keeps the background
# weight buffer fed for pull-ahead. Keep this off.
LDW_OPT = False

if LDW_OPT and not hasattr(_bass_utils, "_orig_run_command_ldw"):
    _bass_utils._orig_run_command_ldw = _bass_utils.run_command

    def _run_command_ldw(cmd, *a, **kw):
        if isinstance(cmd, list):
            cmd = [
                "--enable-ldw-opt=true" if c == "--enable-ldw-opt=false" else c
                for c in cmd
            ]
        return _bass_utils._orig_run_command_ldw(cmd, *a, **kw)

    _bass_utils.run_command = _run_command_ldw

F32 = mybir.dt.float32
F32R = mybir.dt.float32r
BF16 = mybir.dt.bfloat16
RELU = mybir.ActivationFunctionType.Relu
IDENT = mybir.ActivationFunctionType.Identity

N_CORES = 8
TILE = 32  # block-sparse tile size of the masks
P = 128  # partitions

MM_DTYPE = "bf16"  # "f32" | "f32r" | "bf16"


def _build(nc, d_in, d_h, d_out, bc, mm_dtype=MM_DTYPE):
    """Emit the per-core kernel. bc = batch columns per core."""
    kt1 = d_in // P  # k-tiles in layer 1
    mt1 = d_h // P  # m-tiles of h1 (== k-tiles of layer 2)
    mt2 = d_h // P  # m-tiles of h2
    mot = d_out // P  # m-tiles of out
    sw = min(512, bc)  # psum strip width
    ns = bc // sw  # strips per row of tiles

    # storage dtype of mm operands
    sdt = {"bf16": BF16, "f32r": F32R, "f32": F32}[mm_dtype]

    def mm(out_ps, lhsT, rhs, start, stop):
        nc.tensor.matmul(out_ps, lhsT, rhs, start=start, stop=stop)

    xt_d = nc.dram_tensor("xt", [kt1, P, bc], sdt, kind="ExternalInput")
    w1_d = nc.dram_tensor("w1", [mt1, P, d_in], sdt, kind="ExternalInput")
    b1_d = nc.dram_tensor("b1", [P, mt1], F32, kind="ExternalInput")
    w2_d = nc.dram_tensor("w2", [mt2, P, d_h], sdt, kind="ExternalInput")
    b2_d = nc.dram_tensor("b2", [P, mt2], F32, kind="ExternalInput")
    wo_d = nc.dram_tensor("wo", [mt2, P, d_out], sdt, kind="ExternalInput")
    bo_d = nc.dram_tensor("bo", [P, mot], F32, kind="ExternalInput")
    out_d = nc.dram_tensor("out", [mot, P, bc], F32, kind="ExternalOutput")

    with tile.TileContext(nc) as tc:
        with (
            tc.tile_pool(name="bias", bufs=1) as bias_pool,
            tc.tile_pool(name="h1", bufs=1) as h1_pool,
        ):
            b1_sb = bias_pool.tile([P, mt1], F32, tag="b1")
            b2_sb = bias_pool.tile([P, mt2], F32, tag="b2")
            bo_sb = bias_pool.tile([P, mot], F32, tag="bo")
            nc.sync.dma_start(out=b1_sb[:], in_=b1_d[:])
            nc.sync.dma_start(out=b2_sb[:], in_=b2_d[:])
            nc.sync.dma_start(out=bo_sb[:], in_=bo_d[:])

            h1 = []
            for i in range(mt1):
                t = h1_pool.tile([P, bc], sdt, name=f"h1_{i}", tag=f"h1_{i}")
                h1.append(t)

            kh2 = max(mt1 // 8, 1)  # k-tiles per w2 panel piece

            # ps2 lives above the L1 scope so layer 2's first matmuls are not
            # gated on ps1's pool release; w2pre holds the first two pieces of
            # w2's first panel, loaded while layer 1 is still running.
            with (
                tc.tile_pool(name="ps2", bufs=2, space="PSUM") as ps2_pool,
                tc.tile_pool(name="w2pre", bufs=2) as w2pre_pool,
            ):
                # ---------------- Layer 1 ----------------
                with (
                    tc.tile_pool(name="xtp", bufs=1) as xt_pool,
                    tc.tile_pool(name="w1p", bufs=4) as w1_pool,
                    tc.tile_pool(name="ps1", bufs=2, space="PSUM") as ps1_pool,
                ):
                    # stream each weight panel in quarters so the pool stays
                    # small enough to double-buffer within SBUF
                    kh1 = max(kt1 // 4, 1)  # k-tiles per panel piece

                    def load_w1(mt, eng=None):
                        w1h = []
                        for h in range(kt1 // kh1):
                            t = w1_pool.tile([P, kh1 * P], sdt, tag="w1t")
                            (eng or nc.sync).dma_start(
                                out=t[:],
                                in_=w1_d[mt][:, h * kh1 * P : (h + 1) * kh1 * P],
                            )
                            w1h.append(t)
                        return w1h

                    # xt alternates between the sync and gpsimd DMA rings for
                    # 2x startup bandwidth; the first two tiles go at the very
                    # head of each ring so the PE's first matmuls are gated
                    # only on xt_0 + the first w1 piece
                    xt = []
                    for kt in range(kt1):
                        t = xt_pool.tile([P, bc], sdt, name=f"xt_{kt}", tag=f"xt_{kt}")
                        xt.append(t)
                    nc.sync.dma_start(out=xt[0][:], in_=xt_d[0])
                    if kt1 > 1:
                        nc.gpsimd.dma_start(out=xt[1][:], in_=xt_d[1])
                    w1h0 = load_w1(0)
                    for kt in range(2, kt1):
                        eng = nc.sync if kt % 2 == 0 else nc.gpsimd
                        eng.dma_start(out=xt[kt][:], in_=xt_d[kt])

                    w2pre = []
                    for h in range(min(2, mt1 // kh2)):
                        t = w2pre_pool.tile([P, kh2 * P], sdt, tag="w2pre")
                        nc.sync.dma_start(
                            out=t[:], in_=w2_d[0][:, h * kh2 * P : (h + 1) * kh2 * P]
                        )
                        w2pre.append(t)

                    for mt in range(mt1):
                        w1h = w1h0 if mt == 0 else load_w1(mt)
                        # weight-stationary inner order: each lhsT block feeds
                        # every batch strip before the next LDWEIGHTS; the
                        # psum tile spans ns banks so one activation drains it
                        ps = ps1_pool.tile([P, bc], F32, name="ps1", tag="ps1")
                        for kt in range(kt1):
                            h, r = divmod(kt, kh1)
                            for n in range(ns):
                                mm(
                                    ps[:, n * sw : (n + 1) * sw],
                                    w1h[h][:, r * P : (r + 1) * P],
                                    xt[kt][:, n * sw : (n + 1) * sw],
                                    start=(kt == 0),
                                    stop=(kt == kt1 - 1),
                                )
                        nc.scalar.activation(
                            h1[mt][:], ps[:], RELU, bias=b1_sb[:, mt : mt + 1]
                        )

                # ---------------- Layers 2 + 3 (fused) ----------------
                with (
                    tc.tile_pool(name="w2p", bufs=10) as w2_pool,
                    tc.tile_pool(name="wop", bufs=2) as wo_pool,
                    tc.tile_pool(name="h2p", bufs=2) as h2_pool,
                    tc.tile_pool(name="oacc", bufs=1) as oacc_pool,
                    tc.tile_pool(name="ps3", bufs=2, space="PSUM") as ps3_pool,
                ):
                    oacc = []
                    for i in range(mot):
                        t = oacc_pool.tile([P, bc], F32, name=f"oacc_{i}", tag=f"oacc_{i}")
                        oacc.append(t)

                    for mt in range(mt2):
                        if mt == 0:
                            w2h = list(w2pre)
                            hs = len(w2pre)
                        else:
                            w2h = []
                            hs = 0
                        for h in range(hs, mt1 // kh2):
                            t = w2_pool.tile([P, kh2 * P], sdt, tag="w2t")
                            nc.sync.dma_start(
                                out=t[:], in_=w2_d[mt][:, h * kh2 * P : (h + 1) * kh2 * P]
                            )
                            w2h.append(t)
                        wot = wo_pool.tile([P, d_out], sdt, tag="wot")
                        nc.sync.dma_start(out=wot[:], in_=wo_d[mt])
                        h2t = h2_pool.tile([P, bc], sdt, tag="h2t")
                        ps = ps2_pool.tile([P, bc], F32, name="ps2", tag="ps2")
                        for kt in range(mt1):
                            h, r = divmod(kt, kh2)
                            for n in range(ns):
                                mm(
                                    ps[:, n * sw : (n + 1) * sw],
                                    w2h[h][:, r * P : (r + 1) * P],
                                    h1[kt][:, n * sw : (n + 1) * sw],
                                    start=(kt == 0),
                                    stop=(kt == mt1 - 1),
                                )
                        nc.scalar.activation(
                            h2t[:], ps[:], RELU, bias=b2_sb[:, mt : mt + 1]
                        )
                        last = mt == mt2 - 1 and mt2 > 1
                        for mo in range(mot):
                            ps3 = ps3_pool.tile([P, bc], F32, name="ps3", tag="ps3")
                            for n in range(ns):
                                cs = slice(n * sw, (n + 1) * sw)
                                mm(
                                    ps3[:, cs],
                                    wot[:, mo * P : (mo + 1) * P],
                                    h2t[:, cs],
                                    start=True,
                                    stop=True,
                                )
                            if mt == 0:
                                nc.vector.tensor_copy(oacc[mo][:], ps3[:])
                            elif last:
                                # fold the output bias into the final
                                # accumulation: out = (ps3*1 + bo) + oacc
                                nc.vector.affine_then_add(
                                    oacc[mo][:],
                                    ps3[:],
                                    oacc[mo][:],
                                    1.0,
                                    bo_sb[:, mo : mo + 1],
                                )
                            else:
                                nc.vector.tensor_add(oacc[mo][:], oacc[mo][:], ps3[:])
                            if last:
                                nc.sync.dma_start(out=out_d[mo], in_=oacc[mo][:])

                    if mt2 == 1:
                        for mo in range(mot):
                            nc.scalar.activation(
                                oacc[mo][:], oacc[mo][:], IDENT, bias=bo_sb[:, mo : mo + 1]
                            )
                            nc.sync.dma_start(out=out_d[mo], in_=oacc[mo][:])

    nc.compile()
    return nc


def _expand_mask(mask, t=TILE):
    return np.repeat(np.repeat(np.asarray(mask, dtype=bool), t, axis=0), t, axis=1)


def _pack_lhsT(w, d_m, d_k):
    """[d_m, d_k] weights -> [d_m/P, P, d_k] panels.

    panel[mt, i, kt*P + j] = w[mt*P + j, kt*P + i], so each [P, P] slice of a
    panel is a ready-to-use lhsT block (partition dim = contraction dim).
    """
    mt, kt = d_m // P, d_k // P
    return np.ascontiguousarray(
        w.reshape(mt, P, kt, P).transpose(0, 3, 2, 1).reshape(mt, P, d_k)
    )


def _pack_out_panels(w, d_m, d_k):
    """[d_m, d_k] weights -> [d_k/P, P, d_m] panels keyed by the k-tile.

    panel[kt, i, mo*P + j] = w[mo*P + j, kt*P + i].
    """
    mt, kt = d_m // P, d_k // P
    return np.ascontiguousarray(
        w.reshape(mt, P, kt, P).transpose(2, 3, 0, 1).reshape(kt, P, d_m)
    )


def _pack_bias(b):
    n = b.shape[0] // P
    return np.ascontiguousarray(b.reshape(n, P).T)


def _run(
    x,
    w1e,
    b1,
    w2e,
    b2,
    wo,
    bo,
    d_in,
    d_h,
    d_out,
    n_cores=N_CORES,
    trace=False,
    mm_dtype=MM_DTYPE,
):
    b = x.shape[0]
    bc = b // n_cores

    nc = bacc.Bacc(
        "TRN2", target_bir_lowering=False, debug=False, num_devices=n_cores
    )
    _build(nc, d_in, d_h, d_out, bc, mm_dtype=mm_dtype)

    np_sdt = mybir.dt.np(BF16) if mm_dtype == "bf16" else np.float32

    def cvt(a):
        return np.ascontiguousarray(a.astype(np_sdt))

    shared = {
        "w1": cvt(_pack_lhsT(w1e, d_h, d_in)),
        "b1": _pack_bias(b1),
        "w2": cvt(_pack_lhsT(w2e, d_h, d_h)),
        "b2": _pack_bias(b2),
        "wo": cvt(_pack_out_panels(wo, d_out, d_h)),
        "bo": _pack_bias(bo),
    }
    in_maps = []
    for c in range(n_cores):
        xc = np.ascontiguousarray(x[c * bc : (c + 1) * bc].T).reshape(
            d_in // P, P, bc
        )
        in_maps.append({"xt": cvt(xc), **shared})

    res = run_bass_kernel_spmd(
        nc, in_maps, core_ids=list(range(n_cores)), trace=trace
    )
    outs = []
    for c in range(n_cores):
        outs.append(res.results[c]["out"].reshape(d_out, bc))
    full = np.concatenate(outs, axis=1)  # [d_out, B]
    return np.ascontiguousarray(full.T), res


def kernel(x, W1, b1, W2, b2, Wo, bo, mask1, mask2):
    x = np.asarray(x, dtype=np.float32)
    w1e = np.asarray(W1, dtype=np.float32) * _expand_mask(mask1)
    w2e = np.asarray(W2, dtype=np.float32) * _expand_mask(mask2)
    out, _ = _run(
        x,
        w1e,
        np.asarray(b1, np.float32),
        w2e,
        np.asarray(b2, np.float32),
        np.asarray(Wo, np.float32),
        np.asarray(bo, np.float32),
        d_in=2048,
        d_h=4096,
        d_out=1024,
    )
    return out



# revision 14
# speedup vs baseline: 1.0886x; 1.0036x over previous
"""Block-sparse 3-layer MLP on 8 Trainium2 NeuronCores.

Reference computation (fp32):
    h1 = relu(x @ (W1*expand(mask1)).T + b1)       x:[B,2048] W1:[4096,2048]
    h2 = relu(h1 @ (W2*expand(mask2)).T + b2)      W2:[4096,4096]
    out = h2 @ Wo.T + bo                           Wo:[1024,4096] -> [B,1024]

Strategy: data-parallel over the batch (B=8192 -> 1024 rows/core), no
collectives. Masks are applied to the weights on the host (free) and all
matmuls run dense on the PE array: at density 0.5 with 32x32 mask tiles,
skipping zero tiles via PE-array tiling is slower than dense (packed 32x32
tiles reach only ~36% of dense PE throughput), and fp8 DoubleRow (2x PE
rate) fails the 2e-2 error gate (e4m3 one-pass ~6% rel err; an accurate
3-term hi/lo split needs 1.5x the products, i.e. slower than bf16).

All matmul operands are bf16: same 1 cycle/row PE rate as f32r but half
the DMA bytes and SBUF footprint (rel err ~4e-3, budget 2e-2).
Activations are feature-major [features, batch] so biases are
per-partition and lhsT panels are host-pretransposed [128, K] blocks.

Per core, three phases, PE-saturated throughout:
  L1: 32 m-tiles, psum-accumulated over 16 k-tiles, RELU -> h1 resident
      in SBUF (bf16, 8MB). Inputs stream on the three DMA-capable rings
      (sync/gpsimd/scalar) one descriptor per k-tile in PE consumption
      order, critical tiles at the ring heads.
  L2: 32 m-tiles over 32 k-tiles, RELU -> h2 resident (8MB). ps2 opens
      alongside ps1 (4+4 psum banks) so its first accumulation does not
      wait on L1's last psum release.
  L3: output accumulated directly in PSUM: 2 groups of 4 output m-tiles,
      each group's 4 accumulators [128,1024] = 8 psum banks, k-outer over
      the 32 h2 tiles. No SBUF accumulator and no vector adds; the tail
      is the last bias-activation (split scalar/vector) + output DMA
      spread over all three rings.
"""

import sys

sys.path.insert(0, "/opt/trn_rl_repo")

import numpy as np

from concourse import bacc, mybir, tile
from concourse.bass_utils import run_bass_kernel_spmd

F32 = mybir.dt.float32
BF16 = mybir.dt.bfloat16
RELU = mybir.ActivationFunctionType.Relu
IDENT = mybir.ActivationFunctionType.Identity

N_CORES = 8
TILE = 32  # block-sparse tile size of the masks
P = 128  # partitions


def _build(nc, d_in, d_h, d_out, bc):
    """Emit the per-core kernel. bc = batch columns per core."""
    kt1 = d_in // P  # k-tiles in layer 1 (16)
    mt1 = d_h // P  # m-tiles of h1 == k-tiles of layer 2 (32)
    mt2 = d_h // P  # m-tiles of h2 == k-tiles of layer 3 (32)
    mot = d_out // P  # m-tiles of out (8)
    sw = min(512, bc)  # psum strip width
    ns = bc // sw  # strips per row of tiles
    GW = 4  # output m-tiles per L3 psum group (4 x 2 banks = 8 banks)

    xt_d = nc.dram_tensor("xt", [P, kt1 * bc], BF16, kind="ExternalInput")
    w1_d = nc.dram_tensor("w1", [mt1, P, d_in], BF16, kind="ExternalInput")
    b1_d = nc.dram_tensor("b1", [P, mt1], F32, kind="ExternalInput")
    w2_d = nc.dram_tensor("w2", [mt2, P, d_h], BF16, kind="ExternalInput")
    b2_d = nc.dram_tensor("b2", [P, mt2], F32, kind="ExternalInput")
    wo_d = nc.dram_tensor("wo", [mt2, P, d_out], BF16, kind="ExternalInput")
    bo_d = nc.dram_tensor("bo", [P, mot], F32, kind="ExternalInput")
    out_d = nc.dram_tensor("out", [mot, P, bc], F32, kind="ExternalOutput")

    with tile.TileContext(nc) as tc:
        with (
            tc.tile_pool(name="bias", bufs=1) as bias_pool,
            tc.tile_pool(name="h1", bufs=1) as h1_pool,
            tc.tile_pool(name="h2", bufs=1) as h2_pool,
            tc.tile_pool(name="w2p", bufs=3) as w2_pool,
        ):
            b1_sb = bias_pool.tile([P, mt1], F32, tag="b1")
            b2_sb = bias_pool.tile([P, mt2], F32, tag="b2")
            bo_sb = bias_pool.tile([P, mot], F32, tag="bo")

            h1 = []
            h2 = []
            wo_pre = {}
            if True:
                # ---------------- Layer 1 ----------------
                with (
                    tc.tile_pool(name="xtp", bufs=1) as xt_pool,
                    tc.tile_pool(name="w1p", bufs=4) as w1_pool,
                    tc.tile_pool(name="ps1", bufs=2, space="PSUM") as ps1_pool,
                ):
                    xt = xt_pool.tile([P, kt1 * bc], BF16, tag="xt")
                    # Ring heads carry what the first matmuls need: k-tile 0
                    # of x on sync, W1 panel 0 (in quarters, so the PE can
                    # start after 128KB) on gpsimd. xt loads are one
                    # descriptor per k-tile (a DMA completes as a whole)
                    # interleaved over the three rings in PE consumption
                    # order; W1 panels 1-3 and the biases slot in between at
                    # the positions their deadlines allow.
                    rings = [nc.sync, nc.gpsimd, nc.scalar]

                    def load_xt(kt, eng):
                        eng.dma_start(
                            out=xt[:, kt * bc : (kt + 1) * bc],
                            in_=xt_d[:, kt * bc : (kt + 1) * bc],
                        )

                    load_xt(0, nc.sync)
                    w1pre = {}
                    w1t0 = w1_pool.tile([P, d_in], BF16, tag="w1t")
                    q = d_in // 4
                    for i in range(4):
                        nc.gpsimd.dma_start(
                            out=w1t0[:, i * q : (i + 1) * q],
                            in_=w1_d[0][:, i * q : (i + 1) * q],
                        )
                    w1pre[0] = w1t0
                    nc.scalar.dma_start(out=b1_sb[:], in_=b1_d[:])
                    load_xt(1, nc.sync)
                    load_xt(2, nc.scalar)
                    load_xt(3, nc.gpsimd)
                    load_xt(4, nc.sync)
                    load_xt(5, nc.scalar)
                    load_xt(6, nc.gpsimd)
                    for mt in (1, 2, 3):  # W1 panels 1-3 before xt's tail
                        t = w1_pool.tile([P, d_in], BF16, tag="w1t")
                        rings[mt - 1].dma_start(out=t[:], in_=w1_d[mt])
                        w1pre[mt] = t
                    load_xt(7, nc.sync)
                    load_xt(8, nc.scalar)
                    load_xt(9, nc.gpsimd)
                    load_xt(10, nc.sync)
                    load_xt(11, nc.scalar)
                    load_xt(12, nc.gpsimd)
                    load_xt(13, nc.sync)
                    load_xt(14, nc.scalar)
                    load_xt(15, nc.gpsimd)
                    nc.scalar.dma_start(out=b2_sb[:], in_=b2_d[:])
                    nc.scalar.dma_start(out=bo_sb[:], in_=bo_d[:])
                    # W2 panels 0,1 prefetched on the scalar ring during L1
                    w2pre = []
                    for i in range(2):
                        t = w2_pool.tile([P, d_h], BF16, tag="w2t")
                        nc.scalar.dma_start(out=t[:], in_=w2_d[i])
                        w2pre.append(t)

                    for mt in range(mt1):
                        if mt in w1pre:
                            w1t = w1pre[mt]
                        else:
                            w1t = w1_pool.tile([P, d_in], BF16, tag="w1t")
                            eng = nc.sync if mt % 2 else nc.gpsimd
                            eng.dma_start(out=w1t[:], in_=w1_d[mt])
                        ps = ps1_pool.tile([P, bc], F32, tag="ps1")
                        for kt in range(kt1):
                            for n in range(ns):
                                nc.tensor.matmul(
                                    ps[:, n * sw : (n + 1) * sw],
                                    w1t[:, kt * P : (kt + 1) * P],
                                    xt[:, kt * bc + n * sw : kt * bc + (n + 1) * sw],
                                    start=(kt == 0),
                                    stop=(kt == kt1 - 1),
                                )
                        h = h1_pool.tile(
                            [P, bc], BF16, name=f"h1_{mt}", tag=f"h1_{mt}"
                        )
                        nc.scalar.activation(
                            h[:], ps[:], RELU, bias=b1_sb[:, mt : mt + 1]
                        )
                        h1.append(h)

                # ---------------- Layer 2 ----------------
                es_wop = tc.tile_pool(name="wop", bufs=10)
                wo_pool = es_wop.__enter__()
                es_ps2 = tc.tile_pool(name="ps2", bufs=2, space="PSUM")
                ps2_pool = es_ps2.__enter__()
                for mt in range(mt2):
                    if mt < 2:
                        w2t = w2pre[mt]
                    else:
                        w2t = w2_pool.tile([P, d_h], BF16, tag="w2t")
                        eng = nc.sync if mt % 2 else nc.gpsimd
                        eng.dma_start(out=w2t[:], in_=w2_d[mt])
                    ps = ps2_pool.tile([P, bc], F32, tag="ps2")
                    for kt in range(mt1):
                        for n in range(ns):
                            nc.tensor.matmul(
                                ps[:, n * sw : (n + 1) * sw],
                                w2t[:, kt * P : (kt + 1) * P],
                                h1[kt][:, n * sw : (n + 1) * sw],
                                start=(kt == 0),
                                stop=(kt == mt1 - 1),
                            )
                    h = h2_pool.tile([P, bc], BF16, name=f"h2_{mt}", tag=f"h2_{mt}")
                    nc.scalar.activation(h[:], ps[:], RELU, bias=b2_sb[:, mt : mt + 1])
                    h2.append(h)
                    if mt == 24:
                        # L3's first half-panels of Wo on the mostly-idle
                        # scalar ring, well ahead of L3's start
                        for kt in range(4):
                            t = wo_pool.tile([P, GW * P], BF16, tag="wot")
                            nc.scalar.dma_start(out=t[:], in_=wo_d[kt][:, 0 : GW * P])
                            wo_pre[kt] = t

            es_ps2.__exit__(None, None, None)

            # ---------------- Layer 3 ----------------
            # Output accumulates in PSUM across all 32 k-tiles: per group
            # of GW=4 output m-tiles, 4 accumulators of [128, bc] f32
            # occupy all 8 psum banks; Wo half-panels stream k-outer.
            with (
                tc.tile_pool(name="ps3", bufs=1, space="PSUM") as ps3_pool,
                tc.tile_pool(name="osb", bufs=GW) as osb_pool,
            ):
                for g in range(mot // GW):
                    pss = [
                        ps3_pool.tile([P, bc], F32, name=f"ps3_{j}", tag=f"ps3_{j}")
                        for j in range(GW)
                    ]
                    for kt in range(mt2):
                        if g == 0 and kt in wo_pre:
                            wot = wo_pre[kt]
                        else:
                            wot = wo_pool.tile([P, GW * P], BF16, tag="wot")
                            eng = nc.sync if kt % 2 else nc.gpsimd
                            eng.dma_start(
                                out=wot[:],
                                in_=wo_d[kt][:, g * GW * P : (g + 1) * GW * P],
                            )
                        for j in range(GW):
                            for n in range(ns):
                                nc.tensor.matmul(
                                    pss[j][:, n * sw : (n + 1) * sw],
                                    wot[:, j * P : (j + 1) * P],
                                    h2[kt][:, n * sw : (n + 1) * sw],
                                    start=(kt == 0),
                                    stop=(kt == mt2 - 1),
                                )
                    last_g = g == mot // GW - 1
                    for j in range(GW):
                        mo = g * GW + j
                        osb = osb_pool.tile([P, bc], F32, tag="osb")
                        if j == 0 and not last_g:
                            # single full-width op releases this psum
                            # buffer fastest for the next group
                            nc.scalar.activation(
                                osb[:], pss[j][:], IDENT, bias=bo_sb[:, mo : mo + 1]
                            )
                        else:
                            nc.scalar.activation(
                                osb[:, 0:sw],
                                pss[j][:, 0:sw],
                                IDENT,
                                bias=bo_sb[:, mo : mo + 1],
                            )
                            nc.vector.tensor_scalar_add(
                                osb[:, sw:], pss[j][:, sw:], bo_sb[:, mo : mo + 1]
                            )
                        if last_g:
                            # drain the final 2MB over all three rings
                            h_ = sw // 2
                            nc.sync.dma_start(out=out_d[mo][:, 0:h_], in_=osb[:, 0:h_])
                            nc.gpsimd.dma_start(
                                out=out_d[mo][:, h_:sw], in_=osb[:, h_:sw]
                            )
                            nc.scalar.dma_start(
                                out=out_d[mo][:, sw : sw + h_],
                                in_=osb[:, sw : sw + h_],
                            )
                            nc.sync.dma_start(
                                out=out_d[mo][:, sw + h_ :], in_=osb[:, sw + h_ :]
                            )
                        else:
                            nc.sync.dma_start(out=out_d[mo][:, 0:sw], in_=osb[:, 0:sw])
                            nc.gpsimd.dma_start(out=out_d[mo][:, sw:], in_=osb[:, sw:])

            es_wop.__exit__(None, None, None)

    nc.compile()
    return nc


def _expand_mask(mask, t=TILE):
    return np.repeat(np.repeat(np.asarray(mask, dtype=bool), t, axis=0), t, axis=1)


def _pack_lhsT(w, d_m, d_k):
    """[d_m, d_k] weights -> [d_m/P, P, d_k] panels.

    panel[mt, i, kt*P + j] = w[mt*P + j, kt*P + i], so each [P, P] slice of a
    panel is a ready-to-use lhsT block (partition dim = contraction dim).
    """
    mt, kt = d_m // P, d_k // P
    return np.ascontiguousarray(
        w.reshape(mt, P, kt, P).transpose(0, 3, 2, 1).reshape(mt, P, d_k)
    )


def _pack_out_panels(w, d_m, d_k):
    """[d_m, d_k] weights -> [d_k/P, P, d_m] panels keyed by the k-tile.

    panel[kt, i, mo*P + j] = w[mo*P + j, kt*P + i].
    """
    mt, kt = d_m // P, d_k // P
    return np.ascontiguousarray(
        w.reshape(mt, P, kt, P).transpose(2, 3, 0, 1).reshape(kt, P, d_m)
    )


def _pack_bias(b):
    n = b.shape[0] // P
    return np.ascontiguousarray(b.reshape(n, P).T)


def _run(x, w1e, b1, w2e, b2, wo, bo, d_in, d_h, d_out, n_cores=N_CORES, trace=False):
    b = x.shape[0]
    bc = b // n_cores
    kt1 = d_in // P

    nc = bacc.Bacc("TRN2", target_bir_lowering=False, debug=False, num_devices=n_cores)
    _build(nc, d_in, d_h, d_out, bc)

    np_bf16 = mybir.dt.np(BF16)

    def cvt(a):
        return np.ascontiguousarray(a.astype(np_bf16))

    shared = {
        "w1": cvt(_pack_lhsT(w1e, d_h, d_in)),
        "b1": _pack_bias(b1),
        "w2": cvt(_pack_lhsT(w2e, d_h, d_h)),
        "b2": _pack_bias(b2),
        "wo": cvt(_pack_out_panels(wo, d_out, d_h)),
        "bo": _pack_bias(bo),
    }
    in_maps = []
    for c in range(n_cores):
        xc = x[c * bc : (c + 1) * bc]  # [bc, d_in]
        # xt[p, kt*bc + cc] = xc[cc, kt*128 + p]
        xt = xc.T.reshape(kt1, P, bc).transpose(1, 0, 2).reshape(P, kt1 * bc)
        in_maps.append({"xt": cvt(xt), **shared})

    res = run_bass_kernel_spmd(nc, in_maps, core_ids=list(range(n_cores)), trace=trace)
    outs = []
    for c in range(n_cores):
        outs.append(res.results[c]["out"].reshape(d_out, bc))
    full = np.concatenate(outs, axis=1)  # [d_out, B]
    return np.ascontiguousarray(full.T), res


def kernel(x, W1, b1, W2, b2, Wo, bo, mask1, mask2):
    x = np.asarray(x, dtype=np.float32)
    w1e = np.asarray(W1, dtype=np.float32) * _expand_mask(mask1)
    w2e = np.asarray(W2, dtype=np.float32) * _expand_mask(mask2)
    out, _ = _run(
        x,
        w1e,
        np.asarray(b1, np.float32),
        w2e,
        np.asarray(b2, np.float32),
        np.asarray(Wo, np.float32),
        np.asarray(bo, np.float32),
        d_in=2048,
        d_h=4096,
        d_out=1024,
    )
    return out
